# revision 28
# baseline (speedup 1.0000x reference)
"""EquivariantProjectorViaSchur — TRN2 Bass kernel (8 NeuronCores, SPMD).

Math (per 64x64 channel block B of W):
    V   = U_y^T B U_x
    P   = A o V + Bc o V[sig_r][:, sig_c]     (= mask + gather-symmetrize-scatter)
    out = U_y P U_x^T
The masked symmetrization is fused into the PE matmuls via the k-group
structure of the Schur mask (8 rotation groups of 6, 2 parity groups of 8):
    Z[:, o in g]   = (s_g XG_g) @ T1T[:, o in g] + XJ_g @ T1s[:, o in g]
    XG_g = U_x diag(a_g) U_x^T        (symmetric; s_g = 1/2 rot, 1 diag)
    XJ_g[k',q] = 1/2 sum_{k in g} pi_k U_x[k',k] U_x[q, k^1]
    T1T  = (U_y^T B)^T  (produced directly by W-stationary matmuls)
    T1s[q,o] = pi_o * T1T[q, o^1]     (one DVE tensor_mul, paired-reverse AP
                                       times a broadcast [+1,-1] sign tile)
    out  = kron(I2, U_y^T)-contraction of Z^T  (PE transpose + matmul)
Whole datapath is fp16 (PSUM accumulation in f32); host casts W down and the
output back up. Sharding: c_in block-columns — core i owns W[:, i*768:(i+1)*768];
the tiny U/mask-derived factor matrices are replicated (precomputed host-side).
"""
import contextlib
import time

import numpy as np

import concourse.bass as bass
import concourse.tile as tile
import concourse.mybir as mybir
from concourse.tile import ScopedClock

F32 = mybir.dt.float32
F16 = mybir.dt.float16

O = 64
NSTR_CH = 24          # 128-row stripes per b-chunk
NCH = 2               # b chunks of 48 blocks
NQ = 3                # c quads (4 c-blocks = 256 cols each)
NCORE = 8
CSH = 768             # columns per core shard


# ---------------------------------------------------------------------------
# workarounds for this toolchain
# ---------------------------------------------------------------------------
def _patched_drain_and_barrier(self, tick_clock, wait_clock):
    # this walrus build rejects >1 sem-wait on a Drain: split the tail waits
    drain_inst = self.nc.sync.drain()
    wait_clock.add_sem_waits(drain_inst.ins,
                             ScopedClock({None: tick_clock.global_clock}))
    si = drain_inst.ins.sync_info
    waits = list(si.on_wait) if si is not None else []
    if len(waits) > 1:
        drain_inst.ins.sync_info = mybir.SyncInfo(
            on_wait=waits[:1], on_update=list(si.on_update))
        for i in range(1, len(waits)):
            d2 = self.nc.sync.drain()
            d2.ins.sync_info = mybir.SyncInfo(on_wait=[waits[i]], on_update=[])
    self.nc.all_engine_barrier()
    assert self.sems is not None
    popped = self.nc._tile_sem_poison_stack.pop()
    assert popped is self._sem_poison
    self.nc.clear_and_free_semaphores(list(self.sems.allocated().values()))
    self.nc.all_engine_barrier()


tile.TileContext._drain_and_barrier = _patched_drain_and_barrier


def cap_sync_waits(nc):
    """walrus codegen allows only 1 sem-wait per instruction struct here;
    carry the excess on NoOps inserted just before (same engine/point)."""
    for f in nc.m.functions:
        for blk in f.blocks:
            insts = list(blk.instructions)
            out = []
            ctr = 0
            for ins in insts:
                si = ins.sync_info
                waits = list(si.on_wait) if si is not None else []
                if len(waits) > 1:
                    for i in range(len(waits) - 1):
                        n = mybir.InstNoOp(name=f"{ins.name}_w{ctr}",
                                           ins=[], outs=[])
                        ctr += 1
                        n.engine = ins.engine
                        n.sync_info = mybir.SyncInfo(on_wait=[waits[i]],
                                                     on_update=[])
                        out.append(n)
                    ins.sync_info = mybir.SyncInfo(
                        on_wait=waits[-1:], on_update=list(si.on_update))
                out.append(ins)
            blk.instructions = out


# ---------------------------------------------------------------------------
# host-side precompute of the replicated factor matrices
# ---------------------------------------------------------------------------
def host_precompute(U_y, U_x, mask, block_rows, block_cols):
    rows = np.asarray(block_rows); cols = np.asarray(block_cols)
    mask = np.asarray(mask)
    U_y64 = np.asarray(U_y, np.float64); U_x64 = np.asarray(U_x, np.float64)
    r_rot = set(int(x) for x in rows.tolist())
    nqd = len(rows) // 4
    for t in range(nqd):
        r = rows[4 * t:4 * t + 4]; c = cols[4 * t:4 * t + 4]
        assert mask[r, c].all()
        assert r[0] == r[1] and r[2] == r[3] and r[2] == r[0] + 1 and r[0] % 2 == 0
        assert c[0] == c[2] and c[1] == c[3] and c[1] == c[0] + 1 and c[0] % 2 == 0
    groups, seen = [], np.zeros(O, bool)
    for k in range(O):
        if seen[k]:
            continue
        mem = np.where(mask[k] > 0)[0]
        assert (mask[np.ix_(mem, mem)] > 0).all()
        for m in mem:
            seen[m] = True
        groups.append(mem)
    pi = np.where(np.arange(O) % 2 == 0, 1.0, -1.0)
    eye2 = np.eye(2)
    mats, ginfo = [], []
    mats.append(np.kron(eye2, U_y64).astype(np.float32))    # 0: LY (stationary)
    mats.append(np.kron(eye2, U_y64.T).astype(np.float32))  # 1: LS4
    mats.append(np.eye(128, dtype=np.float32))              # 2: identity
    for mem in groups:
        is_R = int(mem[0]) in r_rot
        s = 0.5 if is_R else 1.0
        a = np.zeros(O); a[mem] = 1.0
        XG = s * (U_x64 @ np.diag(a) @ U_x64.T)
        gi_idx = len(mats); mats.append(np.kron(eye2, XG).astype(np.float32))
        ji_idx = None
        if is_R:
            assert len(mem) == (mem[-1] - mem[0] + 1), "rot group not contiguous"
            XJ = np.zeros((O, O))
            for k in mem:
                XJ += 0.5 * pi[k] * np.outer(U_x64[:, k], U_x64[:, k ^ 1])
            ji_idx = len(mats); mats.append(np.kron(eye2, XJ.T).astype(np.float32))
        else:
            st = int(mem[0])
            assert all(int(m) == st + 2 * i for i, m in enumerate(mem)), \
                "diag group not stride-2"
        ginfo.append(dict(mem=[int(x) for x in mem], is_R=is_R,
                          gi=gi_idx, ji=ji_idx))
    sgn = np.zeros((128, 128), np.float32)   # last: [+1,-1] sign pair cols
    sgn[:, 0] = 1.0
    sgn[:, 1] = -1.0
    mats.append(sgn)
    const = np.concatenate(mats, axis=1)
    return np.ascontiguousarray(const.astype(np.float16)), ginfo


class _EvacBalancer:
    """Greedy ACT/DVE/GPSIMD assignment for PSUM->SBUF copies. 16-bit
    src+dst with packed innermost APs hit the DVE 2x_1port mode."""
    def __init__(self, nc):
        self.nc = nc
        self.t_act = 0.0
        self.t_dve = 0.0
        self.t_pool = 0.0

    def copy(self, dst, src, both16=False):
        fd = src.free_size()
        c_act = (280.0 + fd) / 1.2
        c_dve = (160.0 + fd * (0.5 if both16 else 1.0)) / 0.96
        if self.t_act + c_act <= self.t_dve + c_dve:
            self.t_act += c_act
            return self.nc.scalar.copy(dst, src)
        else:
            self.t_dve += c_dve
            return self.nc.vector.tensor_copy(dst, src)


# ---------------------------------------------------------------------------
# device kernel (one program, SPMD over 8 cores)
# ---------------------------------------------------------------------------
def build_kernel(n_const_mats, ginfo):
    nc = bass.Bass("TRN2", target_bir_lowering=False, debug=False,
                   num_devices=1)
    w = nc.dram_tensor("w", [6144, CSH], F16, kind="ExternalInput").ap()
    cst = nc.dram_tensor("cst", [128, n_const_mats * 128], F16,
                         kind="ExternalInput").ap()
    out = nc.dram_tensor("out", [NQ, 12, 128, 1024], F16,
                         kind="ExternalOutput").ap()

    with tile.TileContext(nc) as tc:
        ctx = contextlib.ExitStack()
        with ctx:
            ev = _EvacBalancer(nc)
            csb_p = ctx.enter_context(tc.tile_pool(name="cst", bufs=1))
            wch_p = ctx.enter_context(tc.tile_pool(name="wch", bufs=6))
            t1T_p = ctx.enter_context(tc.tile_pool(name="t1T", bufs=2))
            t1s_p = ctx.enter_context(tc.tile_pool(name="t1s", bufs=1))
            zsb_p = ctx.enter_context(tc.tile_pool(name="zsb", bufs=1))
            ztsb_p = ctx.enter_context(tc.tile_pool(name="ztsb", bufs=1))
            osb_p = ctx.enter_context(tc.tile_pool(name="osb", bufs=4))
            ps_2b = ctx.enter_context(
                tc.tile_pool(name="ps_2b", bufs=3, space="PSUM"))
            ps_tp = ctx.enter_context(
                tc.tile_pool(name="ps_tp", bufs=2, space="PSUM"))

            # constants split: the 3 phase-A/transpose/final mats load first
            # (tiny DMA) so the first W-stationary matmul isn't gated on the
            # full factor-matrix table; the group mats follow the first W load.
            csb1 = csb_p.tile([128, 3 * 128], F16, tag="csb1")
            csb2 = csb_p.tile([128, (n_const_mats - 3) * 128], F16,
                              tag="csb2")
            nc.sync.dma_start(csb1[:], cst[:, 0:3 * 128])

            def cmat(i):
                if i < 3:
                    return csb1[:, i * 128:(i + 1) * 128]
                return csb2[:, (i - 3) * 128:(i - 2) * 128]

            LY, LS4 = cmat(0), cmat(1)
            ident = cmat(2)
            sgn2 = cmat(n_const_mats - 1)[:, 0:2]

            def phase_a_sg(ch, t1T, sg):
                """One 4-stripe section of phase A: W-stationary S1' emits
                T1^T pieces into the consolidated tile [q-part, (qs, s, h.o)]."""
                t1Tv = t1T[:].rearrange("p (qs f) -> p qs f", qs=6)
                wt = wch_p.tile([128, 4 * CSH], F16, tag="w",
                                name=f"w_{ch}_{sg}")
                r0 = (ch * NSTR_CH + sg) * 128
                nc.sync.dma_start(
                    wt[:].rearrange("p (s c) -> p s c", s=4),
                    w[r0:r0 + 512, :].rearrange("(s p) c -> p s c", p=128))
                for qp in range(3):
                    pb = ps_2b.tile([128, 1024], F32, tag="pb", name="pb")
                    for half in range(2):
                        qs = 2 * qp + half
                        for k4 in range(4):
                            lhsT = wt[:, k4 * CSH + qs * 128:
                                      k4 * CSH + (qs + 1) * 128]
                            nc.tensor.matmul(
                                pb[:, half * 512 + k4 * 128:
                                   half * 512 + (k4 + 1) * 128], lhsT, LY)
                    ev.copy(
                        t1Tv[:, 2 * qp:2 * qp + 2, sg * 128:(sg + 4) * 128],
                        pb[:].rearrange("p (h f) -> p h f", h=2))

            def phase_b_q(ch, t1T, q):
                # phase B for one quad: sigma prep, fused group matmuls,
                # transpose, final contraction, store
                tvs4 = t1T[:].rearrange("p (qs b o) -> p qs b o", qs=6, o=64)
                if True:
                    t1s = [t1s_p.tile([128, 48 * 48], F16, tag=f"t1s{cp}",
                                      name=f"t1s_{ch}_{q}_{cp}")
                           for cp in range(2)]
                    for cp in range(2):
                        src = tvs4[:, 2 * q + cp, :, 0:48].rearrange(
                            "p b (pr two) -> p b pr two", two=2)[:, :, :, ::-1]
                        sv = t1s[cp][:].rearrange(
                            "p (b pr two) -> p b pr two", pr=24, two=2)
                        sgb = sgn2.unsqueeze(1).unsqueeze(1).broadcast_to(
                            [128, 48, 24, 2])
                        nc.vector.tensor_mul(sv, src, sgb)
                        ev.t_dve += 1600.0
                    zsb = zsb_p.tile([128, 2 * NSTR_CH * 128], F16,
                                     tag="z", name=f"z_{ch}_{q}")
                    svs = [t1s[cp][:].rearrange("p (b o) -> p b o", o=48)
                           for cp in range(2)]
                    zvs = zsb[:].rearrange("p (cp b o) -> p cp b o",
                                           cp=2, o=64)
                    for g in ginfo:
                        mem = g["mem"]
                        no = 6 if g["is_R"] else 8
                        zp = ps_2b.tile([128, 1024], F32, tag="pb", name="zp")
                        zpv = zp[:].rearrange("p (cp f) -> p cp f", cp=2)
                        for cp in range(2):
                            dst = zpv[:, cp, 0:48 * no]
                            if g["is_R"]:
                                nc.tensor.matmul(
                                    dst, cmat(g["gi"]),
                                    tvs4[:, 2 * q + cp, :,
                                         mem[0]:mem[0] + 6],
                                    start=True, stop=False)
                                nc.tensor.matmul(
                                    dst, cmat(g["ji"]),
                                    svs[cp][:, :, mem[0]:mem[0] + 6],
                                    start=False, stop=True)
                            else:
                                nc.tensor.matmul(
                                    dst, cmat(g["gi"]),
                                    tvs4[:, 2 * q + cp, :, mem[0]:64:2])
                        if g["is_R"]:
                            zdst = zvs[:, :, :, mem[0]:mem[0] + 6]
                        else:
                            zdst = zvs[:, :, :, mem[0]:64:2]
                        ev.copy(zdst, zpv[:, :, 0:48 * no].rearrange(
                            "p cp (b o) -> p cp b o", o=no))
                    zt = ztsb_p.tile([128, NSTR_CH * 256], F16, tag="zt")
                    for jp in range(0, NSTR_CH, 4):
                        pb = ps_tp.tile([128, 1024], F16, tag="tp", name="tp")
                        for k in range(8):
                            j = jp + k // 2
                            cp = k % 2
                            src = zsb[:, cp * NSTR_CH * 128 +
                                      j * 128:cp * NSTR_CH * 128 +
                                      (j + 1) * 128]
                            nc.tensor.transpose(
                                pb[:, k * 128:(k + 1) * 128], src, ident)
                        ev.copy(zt[:, jp * 256:(jp + 4) * 256], pb[:],
                                both16=True)
                    for jq in range(0, NSTR_CH, 4):
                        ob = osb_p.tile([128, 1024], F16, tag="ob")
                        po = ps_2b.tile([128, 1024], F32, tag="pb",
                                        name="po")
                        for h in range(2):
                            jp = jq + 2 * h
                            nc.tensor.matmul(
                                po[:, h * 512:(h + 1) * 512], LS4,
                                zt[:, jp * 256:(jp + 2) * 256])
                        ev.copy(ob[:], po[:])
                        nc.sync.dma_start(out[q, ch * 6 + jq // 4], ob[:])

            # software pipeline: phase A of ch1 is interleaved between the
            # B-phase quads of ch0 so PE/ACT/DVE/DMA stay jointly busy.
            t1T0 = t1T_p.tile([128, 6 * NSTR_CH * 128], F16, tag="t1T",
                              name="t1T_0")
            t1T1 = t1T_p.tile([128, 6 * NSTR_CH * 128], F16, tag="t1T",
                              name="t1T_1")
            phase_a_sg(0, t1T0, 0)
            nc.sync.dma_start(csb2[:], cst[:, 3 * 128:])
            for sg in range(4, NSTR_CH, 4):
                phase_a_sg(0, t1T0, sg)
            for q in range(NQ):
                phase_b_q(0, t1T0, q)
                phase_a_sg(1, t1T1, 8 * q)
                phase_a_sg(1, t1T1, 8 * q + 4)
            for q in range(NQ):
                phase_b_q(1, t1T1, q)
    cap_sync_waits(nc)
    return nc


_CACHE = {}


def kernel(W, U_y, U_x, mask, block_rows, block_cols):
    from concourse import bass_utils

    W = np.asarray(W, np.float32).astype(np.float16)
    const, ginfo = host_precompute(U_y, U_x, mask, block_rows, block_cols)
    n_mats = const.shape[1] // 128

    key = ("nc", n_mats, tuple(tuple(g["mem"]) for g in ginfo))
    if key not in _CACHE:
        _CACHE[key] = build_kernel(n_mats, ginfo)
    nc = _CACHE[key]

    in_maps = []
    for core in range(NCORE):
        Wsh = np.ascontiguousarray(W[:, core * CSH:(core + 1) * CSH])
        in_maps.append({"w": Wsh, "cst": const})

    res = None
    last_exc = None
    for attempt in range(3):
        try:
            res = bass_utils.run_bass_kernel_spmd(
                nc, in_maps, core_ids=list(range(NCORE)))
            break
        except Exception as e:  # transient NRT_EXEC_UNIT states recover
            last_exc = e
            time.sleep(20 * (attempt + 1))
    if res is None:
        raise last_exc
    outs = []
    for core in range(NCORE):
        o3 = np.asarray(res.results[core]["out"], np.float32)
        o = o3.reshape(3, 2, 6, 128, 4, 256).transpose(
            1, 2, 4, 3, 0, 5).reshape(6144, CSH)
        outs.append(o)
    return np.ascontiguousarray(np.concatenate(outs, axis=1))


# revision 29
# speedup vs baseline: 1.0311x; 1.0311x over previous
"""EquivariantProjectorViaSchur — TRN2 Bass kernel (8 NeuronCores, SPMD).

Math (per 64x64 channel block B of W):
    V   = U_y^T B U_x
    P   = A o V + Bc o V[sig_r][:, sig_c]     (= mask + gather-symmetrize-scatter)
    out = U_y P U_x^T
The masked symmetrization is fused into the PE matmuls via the k-group
structure of the Schur mask (8 rotation groups of 6, 2 parity groups of 8):
    Z[:, o in g]   = (s_g XG_g) @ T1T[:, o in g] + XJ_g @ T1s[:, o in g]
    XG_g = U_x diag(a_g) U_x^T        (symmetric; s_g = 1/2 rot, 1 diag)
    XJ_g[k',q] = 1/2 sum_{k in g} pi_k U_x[k',k] U_x[q, k^1]
    T1T  = (U_y^T B)^T  (produced directly by W-stationary matmuls)
    T1s[q,o] = pi_o * T1T[q, o^1]     (one DVE tensor_mul, paired-reverse AP
                                       times a broadcast [+1,-1] sign tile)
    out  = kron(I2, U_y^T)-contraction of Z^T  (PE transpose + matmul)
Whole datapath is fp16 (PSUM accumulation in f32); host casts W down and the
output back up. Sharding: c_in block-columns — core i owns W[:, i*768:(i+1)*768];
the tiny U/mask-derived factor matrices are replicated (precomputed host-side).
"""
import contextlib
import time

import numpy as np

import concourse.bass as bass
import concourse.tile as tile
import concourse.mybir as mybir
from concourse.tile import ScopedClock

F32 = mybir.dt.float32
F16 = mybir.dt.float16

O = 64
NSTR_CH = 24          # 128-row stripes per b-chunk
NCH = 2               # b chunks of 48 blocks
NQ = 3                # c quads (4 c-blocks = 256 cols each)
NCORE = 8
CSH = 768             # columns per core shard


# ---------------------------------------------------------------------------
# workarounds for this toolchain
# ---------------------------------------------------------------------------
def _patched_drain_and_barrier(self, tick_clock, wait_clock):
    # this walrus build rejects >1 sem-wait on a Drain: split the tail waits
    drain_inst = self.nc.sync.drain()
    wait_clock.add_sem_waits(drain_inst.ins,
                             ScopedClock({None: tick_clock.global_clock}))
    si = drain_inst.ins.sync_info
    waits = list(si.on_wait) if si is not None else []
    if len(waits) > 1:
        drain_inst.ins.sync_info = mybir.SyncInfo(
            on_wait=waits[:1], on_update=list(si.on_update))
        for i in range(1, len(waits)):
            d2 = self.nc.sync.drain()
            d2.ins.sync_info = mybir.SyncInfo(on_wait=[waits[i]], on_update=[])
    self.nc.all_engine_barrier()
    assert self.sems is not None
    popped = self.nc._tile_sem_poison_stack.pop()
    assert popped is self._sem_poison
    self.nc.clear_and_free_semaphores(list(self.sems.allocated().values()))
    self.nc.all_engine_barrier()


tile.TileContext._drain_and_barrier = _patched_drain_and_barrier


def cap_sync_waits(nc):
    """walrus codegen allows only 1 sem-wait per instruction struct here;
    carry the excess on NoOps inserted just before (same engine/point)."""
    for f in nc.m.functions:
        for blk in f.blocks:
            insts = list(blk.instructions)
            out = []
            ctr = 0
            for ins in insts:
                si = ins.sync_info
                waits = list(si.on_wait) if si is not None else []
                if len(waits) > 1:
                    for i in range(len(waits) - 1):
                        n = mybir.InstNoOp(name=f"{ins.name}_w{ctr}",
                                           ins=[], outs=[])
                        ctr += 1
                        n.engine = ins.engine
                        n.sync_info = mybir.SyncInfo(on_wait=[waits[i]],
                                                     on_update=[])
                        out.append(n)
                    ins.sync_info = mybir.SyncInfo(
                        on_wait=waits[-1:], on_update=list(si.on_update))
                out.append(ins)
            blk.instructions = out


# ---------------------------------------------------------------------------
# host-side precompute of the replicated factor matrices
# ---------------------------------------------------------------------------
def host_precompute(U_y, U_x, mask, block_rows, block_cols):
    rows = np.asarray(block_rows); cols = np.asarray(block_cols)
    mask = np.asarray(mask)
    U_y64 = np.asarray(U_y, np.float64); U_x64 = np.asarray(U_x, np.float64)
    r_rot = set(int(x) for x in rows.tolist())
    nqd = len(rows) // 4
    for t in range(nqd):
        r = rows[4 * t:4 * t + 4]; c = cols[4 * t:4 * t + 4]
        assert mask[r, c].all()
        assert r[0] == r[1] and r[2] == r[3] and r[2] == r[0] + 1 and r[0] % 2 == 0
        assert c[0] == c[2] and c[1] == c[3] and c[1] == c[0] + 1 and c[0] % 2 == 0
    groups, seen = [], np.zeros(O, bool)
    for k in range(O):
        if seen[k]:
            continue
        mem = np.where(mask[k] > 0)[0]
        assert (mask[np.ix_(mem, mem)] > 0).all()
        for m in mem:
            seen[m] = True
        groups.append(mem)
    pi = np.where(np.arange(O) % 2 == 0, 1.0, -1.0)
    eye2 = np.eye(2)
    mats, ginfo = [], []
    mats.append(np.kron(eye2, U_y64).astype(np.float32))    # 0: LY (stationary)
    mats.append(np.kron(eye2, U_y64.T).astype(np.float32))  # 1: LS4
    mats.append(np.eye(128, dtype=np.float32))              # 2: identity
    for mem in groups:
        is_R = int(mem[0]) in r_rot
        s = 0.5 if is_R else 1.0
        a = np.zeros(O); a[mem] = 1.0
        XG = s * (U_x64 @ np.diag(a) @ U_x64.T)
        gi_idx = len(mats); mats.append(np.kron(eye2, XG).astype(np.float32))
        ji_idx = None
        if is_R:
            assert len(mem) == (mem[-1] - mem[0] + 1), "rot group not contiguous"
            XJ = np.zeros((O, O))
            for k in mem:
                XJ += 0.5 * pi[k] * np.outer(U_x64[:, k], U_x64[:, k ^ 1])
            ji_idx = len(mats); mats.append(np.kron(eye2, XJ.T).astype(np.float32))
        else:
            st = int(mem[0])
            assert all(int(m) == st + 2 * i for i, m in enumerate(mem)), \
                "diag group not stride-2"
        ginfo.append(dict(mem=[int(x) for x in mem], is_R=is_R,
                          gi=gi_idx, ji=ji_idx))
    sgn = np.zeros((128, 128), np.float32)   # last: [+1,-1] sign pair cols
    sgn[:, 0] = 1.0
    sgn[:, 1] = -1.0
    mats.append(sgn)
    const = np.concatenate(mats, axis=1)
    return np.ascontiguousarray(const.astype(np.float16)), ginfo


class _EvacBalancer:
    """Greedy ACT/DVE/GPSIMD assignment for PSUM->SBUF copies. 16-bit
    src+dst with packed innermost APs hit the DVE 2x_1port mode."""
    def __init__(self, nc):
        self.nc = nc
        self.t_act = 0.0
        self.t_dve = 0.0
        self.t_pool = 0.0

    def copy(self, dst, src, both16=False):
        fd = src.free_size()
        c_act = (280.0 + fd) / 1.2
        c_dve = (160.0 + fd * (0.5 if both16 else 1.0)) / 0.96
        if self.t_act + c_act <= self.t_dve + c_dve:
            self.t_act += c_act
            return self.nc.scalar.copy(dst, src)
        else:
            self.t_dve += c_dve
            return self.nc.vector.tensor_copy(dst, src)


# ---------------------------------------------------------------------------
# device kernel (one program, SPMD over 8 cores)
# ---------------------------------------------------------------------------
def build_kernel(n_const_mats, ginfo):
    nc = bass.Bass("TRN2", target_bir_lowering=False, debug=False,
                   num_devices=1)
    w = nc.dram_tensor("w", [6144, CSH], F16, kind="ExternalInput").ap()
    cst = nc.dram_tensor("cst", [128, n_const_mats * 128], F16,
                         kind="ExternalInput").ap()
    out = nc.dram_tensor("out", [NQ, 12, 128, 1024], F16,
                         kind="ExternalOutput").ap()

    with tile.TileContext(nc) as tc:
        ctx = contextlib.ExitStack()
        with ctx:
            ev = _EvacBalancer(nc)
            csb_p = ctx.enter_context(tc.tile_pool(name="cst", bufs=1))
            wch_p = ctx.enter_context(tc.tile_pool(name="wch", bufs=4))
            t1T_p = ctx.enter_context(tc.tile_pool(name="t1T", bufs=2))
            t1s_p = ctx.enter_context(tc.tile_pool(name="t1s", bufs=2))
            zsb_p = ctx.enter_context(tc.tile_pool(name="zsb", bufs=2))
            ztsb_p = ctx.enter_context(tc.tile_pool(name="ztsb", bufs=2))
            osb_p = ctx.enter_context(tc.tile_pool(name="osb", bufs=4))
            ps_2b = ctx.enter_context(
                tc.tile_pool(name="ps_2b", bufs=3, space="PSUM"))
            ps_tp = ctx.enter_context(
                tc.tile_pool(name="ps_tp", bufs=2, space="PSUM"))

            # constants split: the 3 phase-A/transpose/final mats load first
            # (tiny DMA) so the first W-stationary matmul isn't gated on the
            # full factor-matrix table; the group mats follow the first W load.
            csb1 = csb_p.tile([128, 3 * 128], F16, tag="csb1")
            csb2 = csb_p.tile([128, (n_const_mats - 3) * 128], F16,
                              tag="csb2")
            nc.sync.dma_start(csb1[:], cst[:, 0:3 * 128])

            def cmat(i):
                if i < 3:
                    return csb1[:, i * 128:(i + 1) * 128]
                return csb2[:, (i - 3) * 128:(i - 2) * 128]

            LY, LS4 = cmat(0), cmat(1)
            ident = cmat(2)
            sgn2 = cmat(n_const_mats - 1)[:, 0:2]

            def phase_a_sg(ch, t1T, sg):
                """One 4-stripe section of phase A: W-stationary S1' emits
                T1^T pieces into the consolidated tile [q-part, (qs, s, h.o)]."""
                t1Tv = t1T[:].rearrange("p (qs f) -> p qs f", qs=6)
                wt = wch_p.tile([128, 4 * CSH], F16, tag="w",
                                name=f"w_{ch}_{sg}")
                r0 = (ch * NSTR_CH + sg) * 128
                nc.sync.dma_start(
                    wt[:].rearrange("p (s c) -> p s c", s=4),
                    w[r0:r0 + 512, :].rearrange("(s p) c -> p s c", p=128))
                for qp in range(3):
                    pb = ps_2b.tile([128, 1024], F32, tag="pb", name="pb")
                    for half in range(2):
                        qs = 2 * qp + half
                        for k4 in range(4):
                            lhsT = wt[:, k4 * CSH + qs * 128:
                                      k4 * CSH + (qs + 1) * 128]
                            nc.tensor.matmul(
                                pb[:, half * 512 + k4 * 128:
                                   half * 512 + (k4 + 1) * 128], lhsT, LY)
                    ev.copy(
                        t1Tv[:, 2 * qp:2 * qp + 2, sg * 128:(sg + 4) * 128],
                        pb[:].rearrange("p (h f) -> p h f", h=2))

            def phase_b_q(ch, t1T, q):
                # phase B for one quad: sigma prep, fused group matmuls,
                # transpose, final contraction, store
                tvs4 = t1T[:].rearrange("p (qs b o) -> p qs b o", qs=6, o=64)
                if True:
                    t1s = [t1s_p.tile([128, 48 * 48], F16, tag=f"t1s{cp}",
                                      name=f"t1s_{ch}_{q}_{cp}")
                           for cp in range(2)]
                    for cp in range(2):
                        src = tvs4[:, 2 * q + cp, :, 0:48].rearrange(
                            "p b (pr two) -> p b pr two", two=2)[:, :, :, ::-1]
                        sv = t1s[cp][:].rearrange(
                            "p (b pr two) -> p b pr two", pr=24, two=2)
                        sgb = sgn2.unsqueeze(1).unsqueeze(1).broadcast_to(
                            [128, 48, 24, 2])
                        nc.vector.tensor_mul(sv, src, sgb)
                        ev.t_dve += 1600.0
                    zsb = zsb_p.tile([128, 2 * NSTR_CH * 128], F16,
                                     tag="z", name=f"z_{ch}_{q}")
                    svs = [t1s[cp][:].rearrange("p (b o) -> p b o", o=48)
                           for cp in range(2)]
                    zvs = zsb[:].rearrange("p (cp b o) -> p cp b o",
                                           cp=2, o=64)
                    for g in ginfo:
                        mem = g["mem"]
                        no = 6 if g["is_R"] else 8
                        zp = ps_2b.tile([128, 1024], F32, tag="pb", name="zp")
                        zpv = zp[:].rearrange("p (cp f) -> p cp f", cp=2)
                        for cp in range(2):
                            dst = zpv[:, cp, 0:48 * no]
                            if g["is_R"]:
                                nc.tensor.matmul(
                                    dst, cmat(g["gi"]),
                                    tvs4[:, 2 * q + cp, :,
                                         mem[0]:mem[0] + 6],
                                    start=True, stop=False)
                                nc.tensor.matmul(
                                    dst, cmat(g["ji"]),
                                    svs[cp][:, :, mem[0]:mem[0] + 6],
                                    start=False, stop=True)
                            else:
                                nc.tensor.matmul(
                                    dst, cmat(g["gi"]),
                                    tvs4[:, 2 * q + cp, :, mem[0]:64:2])
                        if g["is_R"]:
                            zdst = zvs[:, :, :, mem[0]:mem[0] + 6]
                        else:
                            zdst = zvs[:, :, :, mem[0]:64:2]
                        ev.copy(zdst, zpv[:, :, 0:48 * no].rearrange(
                            "p cp (b o) -> p cp b o", o=no))
                    zt = ztsb_p.tile([128, NSTR_CH * 256], F16, tag="zt")
                    for jp in range(0, NSTR_CH, 4):
                        pb = ps_tp.tile([128, 1024], F16, tag="tp", name="tp")
                        for k in range(8):
                            j = jp + k // 2
                            cp = k % 2
                            src = zsb[:, cp * NSTR_CH * 128 +
                                      j * 128:cp * NSTR_CH * 128 +
                                      (j + 1) * 128]
                            nc.tensor.transpose(
                                pb[:, k * 128:(k + 1) * 128], src, ident)
                        ev.copy(zt[:, jp * 256:(jp + 4) * 256], pb[:],
                                both16=True)
                    for jq in range(0, NSTR_CH, 4):
                        ob = osb_p.tile([128, 1024], F16, tag="ob")
                        po = ps_2b.tile([128, 1024], F32, tag="pb",
                                        name="po")
                        for h in range(2):
                            jp = jq + 2 * h
                            nc.tensor.matmul(
                                po[:, h * 512:(h + 1) * 512], LS4,
                                zt[:, jp * 256:(jp + 2) * 256])
                        ev.copy(ob[:], po[:])
                        nc.sync.dma_start(out[q, ch * 6 + jq // 4], ob[:])

            # software pipeline: phase A of ch1 is interleaved between the
            # B-phase quads of ch0 so PE/ACT/DVE/DMA stay jointly busy.
            t1T0 = t1T_p.tile([128, 6 * NSTR_CH * 128], F16, tag="t1T",
                              name="t1T_0")
            t1T1 = t1T_p.tile([128, 6 * NSTR_CH * 128], F16, tag="t1T",
                              name="t1T_1")
            phase_a_sg(0, t1T0, 0)
            nc.sync.dma_start(csb2[:], cst[:, 3 * 128:])
            for sg in range(4, NSTR_CH, 4):
                phase_a_sg(0, t1T0, sg)
            for q in range(NQ):
                phase_b_q(0, t1T0, q)
                phase_a_sg(1, t1T1, 8 * q)
                phase_a_sg(1, t1T1, 8 * q + 4)
            for q in range(NQ):
                phase_b_q(1, t1T1, q)
    cap_sync_waits(nc)
    return nc


_CACHE = {}


def kernel(W, U_y, U_x, mask, block_rows, block_cols):
    from concourse import bass_utils

    W = np.asarray(W, np.float32).astype(np.float16)
    const, ginfo = host_precompute(U_y, U_x, mask, block_rows, block_cols)
    n_mats = const.shape[1] // 128

    key = ("nc", n_mats, tuple(tuple(g["mem"]) for g in ginfo))
    if key not in _CACHE:
        _CACHE[key] = build_kernel(n_mats, ginfo)
    nc = _CACHE[key]

    in_maps = []
    for core in range(NCORE):
        Wsh = np.ascontiguousarray(W[:, core * CSH:(core + 1) * CSH])
        in_maps.append({"w": Wsh, "cst": const})

    res = None
    last_exc = None
    for attempt in range(3):
        try:
            res = bass_utils.run_bass_kernel_spmd(
                nc, in_maps, core_ids=list(range(NCORE)))
            break
        except Exception as e:  # transient NRT_EXEC_UNIT states recover
            last_exc = e
            time.sleep(20 * (attempt + 1))
    if res is None:
        raise last_exc
    outs = []
    for core in range(NCORE):
        o3 = np.asarray(res.results[core]["out"], np.float32)
        o = o3.reshape(3, 2, 6, 128, 4, 256).transpose(
            1, 2, 4, 3, 0, 5).reshape(6144, CSH)
        outs.append(o)
    return np.ascontiguousarray(np.concatenate(outs, axis=1))


# revision 31
# speedup vs baseline: 1.0450x; 1.0135x over previous
"""EquivariantProjectorViaSchur — TRN2 Bass kernel (8 NeuronCores, SPMD).

Math (per 64x64 channel block B of W):
    V   = U_y^T B U_x
    P   = A o V + Bc o V[sig_r][:, sig_c]     (= mask + gather-symmetrize-scatter)
    out = U_y P U_x^T
The masked symmetrization is fused into the PE matmuls via the k-group
structure of the Schur mask (8 rotation groups of 6, 2 parity groups of 8):
    Z[:, o in g]   = (s_g XG_g) @ T1T[:, o in g] + XJ_g @ T1s[:, o in g]
    XG_g = U_x diag(a_g) U_x^T        (symmetric; s_g = 1/2 rot, 1 diag)
    XJ_g[k',q] = 1/2 sum_{k in g} pi_k U_x[k',k] U_x[q, k^1]
    T1T  = (U_y^T B)^T  (produced directly by W-stationary matmuls)
    T1s[q,o] = pi_o * T1T[q, o^1]     (one DVE tensor_mul, paired-reverse AP
                                       times a broadcast [+1,-1] sign tile)
    out  = kron(I2, U_y^T)-contraction of Z^T  (PE transpose + matmul)
Whole datapath is fp16 (PSUM accumulation in f32); host casts W down and the
output back up. Sharding: c_in block-columns — core i owns W[:, i*768:(i+1)*768];
the tiny U/mask-derived factor matrices are replicated (precomputed host-side).
"""
import contextlib
import time

import numpy as np

import concourse.bass as bass
import concourse.tile as tile
import concourse.mybir as mybir
from concourse.tile import ScopedClock

F32 = mybir.dt.float32
F16 = mybir.dt.float16

O = 64
NSTR_CH = 24          # 128-row stripes per b-chunk
NCH = 2               # b chunks of 48 blocks
NQ = 3                # c quads (4 c-blocks = 256 cols each)
NCORE = 8
CSH = 768             # columns per core shard


# ---------------------------------------------------------------------------
# workarounds for this toolchain
# ---------------------------------------------------------------------------
def _patched_drain_and_barrier(self, tick_clock, wait_clock):
    # this walrus build rejects >1 sem-wait on a Drain: split the tail waits
    drain_inst = self.nc.sync.drain()
    wait_clock.add_sem_waits(drain_inst.ins,
                             ScopedClock({None: tick_clock.global_clock}))
    si = drain_inst.ins.sync_info
    waits = list(si.on_wait) if si is not None else []
    if len(waits) > 1:
        drain_inst.ins.sync_info = mybir.SyncInfo(
            on_wait=waits[:1], on_update=list(si.on_update))
        for i in range(1, len(waits)):
            d2 = self.nc.sync.drain()
            d2.ins.sync_info = mybir.SyncInfo(on_wait=[waits[i]], on_update=[])
    self.nc.all_engine_barrier()
    assert self.sems is not None
    popped = self.nc._tile_sem_poison_stack.pop()
    assert popped is self._sem_poison
    self.nc.clear_and_free_semaphores(list(self.sems.allocated().values()))
    self.nc.all_engine_barrier()


tile.TileContext._drain_and_barrier = _patched_drain_and_barrier


def cap_sync_waits(nc):
    """walrus codegen allows only 1 sem-wait per instruction struct here;
    carry the excess on NoOps inserted just before (same engine/point)."""
    for f in nc.m.functions:
        for blk in f.blocks:
            insts = list(blk.instructions)
            out = []
            ctr = 0
            for ins in insts:
                si = ins.sync_info
                waits = list(si.on_wait) if si is not None else []
                if len(waits) > 1:
                    for i in range(len(waits) - 1):
                        n = mybir.InstNoOp(name=f"{ins.name}_w{ctr}",
                                           ins=[], outs=[])
                        ctr += 1
                        n.engine = ins.engine
                        n.sync_info = mybir.SyncInfo(on_wait=[waits[i]],
                                                     on_update=[])
                        out.append(n)
                    ins.sync_info = mybir.SyncInfo(
                        on_wait=waits[-1:], on_update=list(si.on_update))
                out.append(ins)
            blk.instructions = out


# ---------------------------------------------------------------------------
# host-side precompute of the replicated factor matrices
# ---------------------------------------------------------------------------
def host_precompute(U_y, U_x, mask, block_rows, block_cols):
    rows = np.asarray(block_rows); cols = np.asarray(block_cols)
    mask = np.asarray(mask)
    U_y64 = np.asarray(U_y, np.float64); U_x64 = np.asarray(U_x, np.float64)
    r_rot = set(int(x) for x in rows.tolist())
    nqd = len(rows) // 4
    for t in range(nqd):
        r = rows[4 * t:4 * t + 4]; c = cols[4 * t:4 * t + 4]
        assert mask[r, c].all()
        assert r[0] == r[1] and r[2] == r[3] and r[2] == r[0] + 1 and r[0] % 2 == 0
        assert c[0] == c[2] and c[1] == c[3] and c[1] == c[0] + 1 and c[0] % 2 == 0
    groups, seen = [], np.zeros(O, bool)
    for k in range(O):
        if seen[k]:
            continue
        mem = np.where(mask[k] > 0)[0]
        assert (mask[np.ix_(mem, mem)] > 0).all()
        for m in mem:
            seen[m] = True
        groups.append(mem)
    pi = np.where(np.arange(O) % 2 == 0, 1.0, -1.0)
    eye2 = np.eye(2)
    mats, ginfo = [], []
    mats.append(np.kron(eye2, U_y64).astype(np.float32))    # 0: LY (stationary)
    mats.append(np.kron(eye2, U_y64.T).astype(np.float32))  # 1: LS4
    mats.append(np.eye(128, dtype=np.float32))              # 2: identity
    for mem in groups:
        is_R = int(mem[0]) in r_rot
        s = 0.5 if is_R else 1.0
        a = np.zeros(O); a[mem] = 1.0
        XG = s * (U_x64 @ np.diag(a) @ U_x64.T)
        gi_idx = len(mats); mats.append(np.kron(eye2, XG).astype(np.float32))
        ji_idx = None
        if is_R:
            assert len(mem) == (mem[-1] - mem[0] + 1), "rot group not contiguous"
            XJ = np.zeros((O, O))
            for k in mem:
                XJ += 0.5 * pi[k] * np.outer(U_x64[:, k], U_x64[:, k ^ 1])
            ji_idx = len(mats); mats.append(np.kron(eye2, XJ.T).astype(np.float32))
        else:
            st = int(mem[0])
            assert all(int(m) == st + 2 * i for i, m in enumerate(mem)), \
                "diag group not stride-2"
        ginfo.append(dict(mem=[int(x) for x in mem], is_R=is_R,
                          gi=gi_idx, ji=ji_idx))
    sgn = np.zeros((128, 128), np.float32)   # last: [+1,-1] sign pair cols
    sgn[:, 0] = 1.0
    sgn[:, 1] = -1.0
    mats.append(sgn)
    const = np.concatenate(mats, axis=1)
    return np.ascontiguousarray(const.astype(np.float16)), ginfo


class _EvacBalancer:
    """Greedy ACT/DVE/GPSIMD assignment for PSUM->SBUF copies. 16-bit
    src+dst with packed innermost APs hit the DVE 2x_1port mode."""
    def __init__(self, nc):
        self.nc = nc
        self.t_act = 0.0
        self.t_dve = 0.0
        self.t_pool = 0.0

    def copy(self, dst, src, both16=False):
        fd = src.free_size()
        c_act = (280.0 + fd) / 1.2
        c_dve = (160.0 + fd * (0.5 if both16 else 1.0)) / 0.96
        if self.t_act + c_act <= self.t_dve + c_dve:
            self.t_act += c_act
            return self.nc.scalar.copy(dst, src)
        else:
            self.t_dve += c_dve
            return self.nc.vector.tensor_copy(dst, src)


# ---------------------------------------------------------------------------
# device kernel (one program, SPMD over 8 cores)
# ---------------------------------------------------------------------------
def build_kernel(n_const_mats, ginfo):
    nc = bass.Bass("TRN2", target_bir_lowering=False, debug=False,
                   num_devices=1)
    w = nc.dram_tensor("w", [6144, CSH], F16, kind="ExternalInput").ap()
    cst = nc.dram_tensor("cst", [128, n_const_mats * 128], F16,
                         kind="ExternalInput").ap()
    out = nc.dram_tensor("out", [NQ, 12, 128, 1024], F16,
                         kind="ExternalOutput").ap()

    with tile.TileContext(nc) as tc:
        ctx = contextlib.ExitStack()
        with ctx:
            ev = _EvacBalancer(nc)
            csb_p = ctx.enter_context(tc.tile_pool(name="cst", bufs=1))
            wch_p = ctx.enter_context(tc.tile_pool(name="wch", bufs=4))
            t1T_p = ctx.enter_context(tc.tile_pool(name="t1T", bufs=2))
            t1s_p = ctx.enter_context(tc.tile_pool(name="t1s", bufs=2))
            zsb_p = ctx.enter_context(tc.tile_pool(name="zsb", bufs=2))
            ztsb_p = ctx.enter_context(tc.tile_pool(name="ztsb", bufs=2))
            osb_p = ctx.enter_context(tc.tile_pool(name="osb", bufs=4))
            ps_2b = ctx.enter_context(
                tc.tile_pool(name="ps_2b", bufs=3, space="PSUM"))
            ps_tp = ctx.enter_context(
                tc.tile_pool(name="ps_tp", bufs=2, space="PSUM"))

            # constants split: the 3 phase-A/transpose/final mats load first
            # (tiny DMA) so the first W-stationary matmul isn't gated on the
            # full factor-matrix table; the group mats follow the first W load.
            csb1 = csb_p.tile([128, 3 * 128], F16, tag="csb1")
            csb2 = csb_p.tile([128, (n_const_mats - 3) * 128], F16,
                              tag="csb2")
            nc.sync.dma_start(csb1[:], cst[:, 0:3 * 128])

            def cmat(i):
                if i < 3:
                    return csb1[:, i * 128:(i + 1) * 128]
                return csb2[:, (i - 3) * 128:(i - 2) * 128]

            LY, LS4 = cmat(0), cmat(1)
            ident = cmat(2)
            sgn2 = cmat(n_const_mats - 1)[:, 0:2]

            def phase_a_sg(ch, t1T, sg):
                """One 4-stripe section of phase A: W-stationary S1' emits
                T1^T pieces into the consolidated tile [q-part, (qs, s, h.o)]."""
                t1Tv = t1T[:].rearrange("p (qs f) -> p qs f", qs=6)
                wt = wch_p.tile([128, 4 * CSH], F16, tag="w",
                                name=f"w_{ch}_{sg}")
                r0 = (ch * NSTR_CH + sg) * 128
                nc.sync.dma_start(
                    wt[:].rearrange("p (s c) -> p s c", s=4),
                    w[r0:r0 + 512, :].rearrange("(s p) c -> p s c", p=128))
                for qp in range(3):
                    pb = ps_2b.tile([128, 1024], F32, tag="pb", name="pb")
                    for half in range(2):
                        qs = 2 * qp + half
                        for k4 in range(4):
                            lhsT = wt[:, k4 * CSH + qs * 128:
                                      k4 * CSH + (qs + 1) * 128]
                            nc.tensor.matmul(
                                pb[:, half * 512 + k4 * 128:
                                   half * 512 + (k4 + 1) * 128], lhsT, LY)
                    ev.copy(
                        t1Tv[:, 2 * qp:2 * qp + 2, sg * 128:(sg + 4) * 128],
                        pb[:].rearrange("p (h f) -> p h f", h=2))

            def phase_b_q(ch, t1T, q):
                # phase B for one quad: sigma prep, fused group matmuls,
                # transpose, final contraction, store
                tvs4 = t1T[:].rearrange("p (qs b o) -> p qs b o", qs=6, o=64)
                if True:
                    t1s = [t1s_p.tile([128, 48 * 48], F16, tag=f"t1s{cp}",
                                      name=f"t1s_{ch}_{q}_{cp}")
                           for cp in range(2)]
                    for cp in range(2):
                        src = tvs4[:, 2 * q + cp, :, 0:48].rearrange(
                            "p b (pr two) -> p b pr two", two=2)[:, :, :, ::-1]
                        sv = t1s[cp][:].rearrange(
                            "p (b pr two) -> p b pr two", pr=24, two=2)
                        sgb = sgn2.unsqueeze(1).unsqueeze(1).broadcast_to(
                            [128, 48, 24, 2])
                        nc.vector.tensor_mul(sv, src, sgb)
                        ev.t_dve += 1600.0
                    zsb = zsb_p.tile([128, 2 * NSTR_CH * 128], F16,
                                     tag="z", name=f"z_{ch}_{q}")
                    svs = [t1s[cp][:].rearrange("p (b o) -> p b o", o=48)
                           for cp in range(2)]
                    zvs = zsb[:].rearrange("p (cp b o) -> p cp b o",
                                           cp=2, o=64)
                    for g in ginfo:
                        mem = g["mem"]
                        no = 6 if g["is_R"] else 8
                        zp = ps_2b.tile([128, 1024], F32, tag="pb", name="zp")
                        zpv = zp[:].rearrange("p (cp f) -> p cp f", cp=2)
                        for cp in range(2):
                            dst = zpv[:, cp, 0:48 * no]
                            if g["is_R"]:
                                nc.tensor.matmul(
                                    dst, cmat(g["gi"]),
                                    tvs4[:, 2 * q + cp, :,
                                         mem[0]:mem[0] + 6],
                                    start=True, stop=False)
                                nc.tensor.matmul(
                                    dst, cmat(g["ji"]),
                                    svs[cp][:, :, mem[0]:mem[0] + 6],
                                    start=False, stop=True)
                            else:
                                nc.tensor.matmul(
                                    dst, cmat(g["gi"]),
                                    tvs4[:, 2 * q + cp, :, mem[0]:64:2])
                        if g["is_R"]:
                            zdst = zvs[:, :, :, mem[0]:mem[0] + 6]
                        else:
                            zdst = zvs[:, :, :, mem[0]:64:2]
                        ev.copy(zdst, zpv[:, :, 0:48 * no].rearrange(
                            "p cp (b o) -> p cp b o", o=no))
                    zt = ztsb_p.tile([128, NSTR_CH * 256], F16, tag="zt")
                    for jp in range(0, NSTR_CH, 4):
                        pb = ps_tp.tile([128, 1024], F16, tag="tp", name="tp")
                        for k in range(8):
                            j = jp + k // 2
                            cp = k % 2
                            src = zsb[:, cp * NSTR_CH * 128 +
                                      j * 128:cp * NSTR_CH * 128 +
                                      (j + 1) * 128]
                            nc.tensor.transpose(
                                pb[:, k * 128:(k + 1) * 128], src, ident)
                        ev.copy(zt[:, jp * 256:(jp + 4) * 256], pb[:],
                                both16=True)
                    for jq in range(0, NSTR_CH, 4):
                        ob = osb_p.tile([128, 1024], F16, tag="ob")
                        po = ps_2b.tile([128, 1024], F32, tag="pb",
                                        name="po")
                        for h in range(2):
                            jp = jq + 2 * h
                            nc.tensor.matmul(
                                po[:, h * 512:(h + 1) * 512], LS4,
                                zt[:, jp * 256:(jp + 2) * 256])
                        ev.copy(ob[:], po[:])
                        nc.sync.dma_start(out[q, ch * 6 + jq // 4], ob[:])

            # software pipeline: phase A of ch1 is interleaved between the
            # B-phase quads of ch0 so PE/ACT/DVE/DMA stay jointly busy.
            t1T0 = t1T_p.tile([128, 6 * NSTR_CH * 128], F16, tag="t1T",
                              name="t1T_0")
            t1T1 = t1T_p.tile([128, 6 * NSTR_CH * 128], F16, tag="t1T",
                              name="t1T_1")
            phase_a_sg(0, t1T0, 0)
            nc.sync.dma_start(csb2[:], cst[:, 3 * 128:])
            for sg in range(4, NSTR_CH, 4):
                phase_a_sg(0, t1T0, sg)
            INTERLEAVE = False
            if INTERLEAVE:
                for q in range(NQ):
                    phase_b_q(0, t1T0, q)
                    phase_a_sg(1, t1T1, 8 * q)
                    phase_a_sg(1, t1T1, 8 * q + 4)
                for q in range(NQ):
                    phase_b_q(1, t1T1, q)
            else:
                for sg in range(0, NSTR_CH, 4):
                    phase_a_sg(1, t1T1, sg)
                for q in range(NQ):
                    phase_b_q(0, t1T0, q)
                for q in range(NQ):
                    phase_b_q(1, t1T1, q)
    cap_sync_waits(nc)
    return nc


_CACHE = {}


def kernel(W, U_y, U_x, mask, block_rows, block_cols):
    from concourse import bass_utils

    W = np.asarray(W, np.float32).astype(np.float16)
    const, ginfo = host_precompute(U_y, U_x, mask, block_rows, block_cols)
    n_mats = const.shape[1] // 128

    key = ("nc", n_mats, tuple(tuple(g["mem"]) for g in ginfo))
    if key not in _CACHE:
        _CACHE[key] = build_kernel(n_mats, ginfo)
    nc = _CACHE[key]

    in_maps = []
    for core in range(NCORE):
        Wsh = np.ascontiguousarray(W[:, core * CSH:(core + 1) * CSH])
        in_maps.append({"w": Wsh, "cst": const})

    res = None
    last_exc = None
    for attempt in range(3):
        try:
            res = bass_utils.run_bass_kernel_spmd(
                nc, in_maps, core_ids=list(range(NCORE)))
            break
        except Exception as e:  # transient NRT_EXEC_UNIT states recover
            last_exc = e
            time.sleep(20 * (attempt + 1))
    if res is None:
        raise last_exc
    outs = []
    for core in range(NCORE):
        o3 = np.asarray(res.results[core]["out"], np.float32)
        o = o3.reshape(3, 2, 6, 128, 4, 256).transpose(
            1, 2, 4, 3, 0, 5).reshape(6144, CSH)
        outs.append(o)
    return np.ascontiguousarray(np.concatenate(outs, axis=1))


# revision 32
# speedup vs baseline: 1.1014x; 1.0540x over previous
"""EquivariantProjectorViaSchur — TRN2 Bass kernel (8 NeuronCores, SPMD).

Math (per 64x64 channel block B of W):
    V   = U_y^T B U_x
    P   = A o V + Bc o V[sig_r][:, sig_c]     (= mask + gather-symmetrize-scatter)
    out = U_y P U_x^T
The masked symmetrization is fused into the PE matmuls via the k-group
structure of the Schur mask (8 rotation groups of 6, 2 parity groups of 8):
    Z[:, o in g]   = (s_g XG_g) @ T1T[:, o in g] + XJ_g @ T1s[:, o in g]
    XG_g = U_x diag(a_g) U_x^T        (symmetric; s_g = 1/2 rot, 1 diag)
    XJ_g[k',q] = 1/2 sum_{k in g} pi_k U_x[k',k] U_x[q, k^1]
    T1T  = (U_y^T B)^T  (produced directly by W-stationary matmuls)
    T1s[q,o] = pi_o * T1T[q, o^1]     (one DVE tensor_mul, paired-reverse AP
                                       times a broadcast [+1,-1] sign tile)
    out  = kron(I2, U_y^T)-contraction of Z^T  (PE transpose + matmul)
Whole datapath is fp16 (PSUM accumulation in f32); host casts W down and the
output back up. Sharding: c_in block-columns — core i owns W[:, i*768:(i+1)*768];
the tiny U/mask-derived factor matrices are replicated (precomputed host-side).
"""
import contextlib
import time

import numpy as np

import concourse.bass as bass
import concourse.tile as tile
import concourse.mybir as mybir
from concourse.tile import ScopedClock

F32 = mybir.dt.float32
F16 = mybir.dt.float16

O = 64
NSTR_CH = 24          # 128-row stripes per b-chunk
NCH = 2               # b chunks of 48 blocks
NQ = 3                # c quads (4 c-blocks = 256 cols each)
NCORE = 8
CSH = 768             # columns per core shard


# ---------------------------------------------------------------------------
# workarounds for this toolchain
# ---------------------------------------------------------------------------
def _patched_drain_and_barrier(self, tick_clock, wait_clock):
    # this walrus build rejects >1 sem-wait on a Drain: split the tail waits
    drain_inst = self.nc.sync.drain()
    wait_clock.add_sem_waits(drain_inst.ins,
                             ScopedClock({None: tick_clock.global_clock}))
    si = drain_inst.ins.sync_info
    waits = list(si.on_wait) if si is not None else []
    if len(waits) > 1:
        drain_inst.ins.sync_info = mybir.SyncInfo(
            on_wait=waits[:1], on_update=list(si.on_update))
        for i in range(1, len(waits)):
            d2 = self.nc.sync.drain()
            d2.ins.sync_info = mybir.SyncInfo(on_wait=[waits[i]], on_update=[])
    self.nc.all_engine_barrier()
    assert self.sems is not None
    popped = self.nc._tile_sem_poison_stack.pop()
    assert popped is self._sem_poison
    self.nc.clear_and_free_semaphores(list(self.sems.allocated().values()))
    self.nc.all_engine_barrier()


tile.TileContext._drain_and_barrier = _patched_drain_and_barrier


def cap_sync_waits(nc):
    """walrus codegen allows only 1 sem-wait per instruction struct here;
    carry the excess on NoOps inserted just before (same engine/point)."""
    for f in nc.m.functions:
        for blk in f.blocks:
            insts = list(blk.instructions)
            out = []
            ctr = 0
            for ins in insts:
                si = ins.sync_info
                waits = list(si.on_wait) if si is not None else []
                if len(waits) > 1:
                    for i in range(len(waits) - 1):
                        n = mybir.InstNoOp(name=f"{ins.name}_w{ctr}",
                                           ins=[], outs=[])
                        ctr += 1
                        n.engine = ins.engine
                        n.sync_info = mybir.SyncInfo(on_wait=[waits[i]],
                                                     on_update=[])
                        out.append(n)
                    ins.sync_info = mybir.SyncInfo(
                        on_wait=waits[-1:], on_update=list(si.on_update))
                out.append(ins)
            blk.instructions = out


# ---------------------------------------------------------------------------
# host-side precompute of the replicated factor matrices
# ---------------------------------------------------------------------------
def host_precompute(U_y, U_x, mask, block_rows, block_cols):
    rows = np.asarray(block_rows); cols = np.asarray(block_cols)
    mask = np.asarray(mask)
    U_y64 = np.asarray(U_y, np.float64); U_x64 = np.asarray(U_x, np.float64)
    r_rot = set(int(x) for x in rows.tolist())
    nqd = len(rows) // 4
    for t in range(nqd):
        r = rows[4 * t:4 * t + 4]; c = cols[4 * t:4 * t + 4]
        assert mask[r, c].all()
        assert r[0] == r[1] and r[2] == r[3] and r[2] == r[0] + 1 and r[0] % 2 == 0
        assert c[0] == c[2] and c[1] == c[3] and c[1] == c[0] + 1 and c[0] % 2 == 0
    groups, seen = [], np.zeros(O, bool)
    for k in range(O):
        if seen[k]:
            continue
        mem = np.where(mask[k] > 0)[0]
        assert (mask[np.ix_(mem, mem)] > 0).all()
        for m in mem:
            seen[m] = True
        groups.append(mem)
    pi = np.where(np.arange(O) % 2 == 0, 1.0, -1.0)
    eye2 = np.eye(2)
    mats, ginfo = [], []
    mats.append(np.kron(eye2, U_y64).astype(np.float32))    # 0: LY (stationary)
    mats.append(np.kron(eye2, U_y64.T).astype(np.float32))  # 1: LS4
    mats.append(np.eye(128, dtype=np.float32))              # 2: identity
    for mem in groups:
        is_R = int(mem[0]) in r_rot
        s = 0.5 if is_R else 1.0
        a = np.zeros(O); a[mem] = 1.0
        XG = s * (U_x64 @ np.diag(a) @ U_x64.T)
        gi_idx = len(mats); mats.append(np.kron(eye2, XG).astype(np.float32))
        ji_idx = None
        if is_R:
            assert len(mem) == (mem[-1] - mem[0] + 1), "rot group not contiguous"
            XJ = np.zeros((O, O))
            for k in mem:
                XJ += 0.5 * pi[k] * np.outer(U_x64[:, k], U_x64[:, k ^ 1])
            ji_idx = len(mats); mats.append(np.kron(eye2, XJ.T).astype(np.float32))
        else:
            st = int(mem[0])
            assert all(int(m) == st + 2 * i for i, m in enumerate(mem)), \
                "diag group not stride-2"
        ginfo.append(dict(mem=[int(x) for x in mem], is_R=is_R,
                          gi=gi_idx, ji=ji_idx))
    sgn = np.zeros((128, 128), np.float32)   # last: [+1,-1] sign pair cols
    sgn[:, 0] = 1.0
    sgn[:, 1] = -1.0
    mats.append(sgn)
    const = np.concatenate(mats, axis=1)
    return np.ascontiguousarray(const.astype(np.float16)), ginfo


class _EvacBalancer:
    """Greedy ACT/DVE/GPSIMD assignment for PSUM->SBUF copies. 16-bit
    src+dst with packed innermost APs hit the DVE 2x_1port mode."""
    def __init__(self, nc):
        self.nc = nc
        self.t_act = 0.0
        self.t_dve = 0.0
        self.t_pool = 0.0

    def copy(self, dst, src, both16=False):
        fd = src.free_size()
        c_act = (280.0 + fd) / 1.2
        c_dve = (160.0 + fd * (0.5 if both16 else 1.0)) / 0.96
        if self.t_act + c_act <= self.t_dve + c_dve:
            self.t_act += c_act
            return self.nc.scalar.copy(dst, src)
        else:
            self.t_dve += c_dve
            return self.nc.vector.tensor_copy(dst, src)


# ---------------------------------------------------------------------------
# device kernel (one program, SPMD over 8 cores)
# ---------------------------------------------------------------------------
def build_kernel(n_const_mats, ginfo):
    nc = bass.Bass("TRN2", target_bir_lowering=False, debug=False,
                   num_devices=1)
    w = nc.dram_tensor("w", [6144, CSH], F16, kind="ExternalInput").ap()
    cst = nc.dram_tensor("cst", [128, n_const_mats * 128], F16,
                         kind="ExternalInput").ap()
    out = nc.dram_tensor("out", [NQ, 12, 128, 1024], F16,
                         kind="ExternalOutput").ap()

    with tile.TileContext(nc) as tc:
        ctx = contextlib.ExitStack()
        with ctx:
            ev = _EvacBalancer(nc)
            csb_p = ctx.enter_context(tc.tile_pool(name="cst", bufs=1))
            wch_p = ctx.enter_context(tc.tile_pool(name="wch", bufs=4))
            t1T_p = ctx.enter_context(tc.tile_pool(name="t1T", bufs=2))
            t1s_p = ctx.enter_context(tc.tile_pool(name="t1s", bufs=2))
            zsb_p = ctx.enter_context(tc.tile_pool(name="zsb", bufs=2))
            ztsb_p = ctx.enter_context(tc.tile_pool(name="ztsb", bufs=2))
            osb_p = ctx.enter_context(tc.tile_pool(name="osb", bufs=4))
            ps_2b = ctx.enter_context(
                tc.tile_pool(name="ps_2b", bufs=3, space="PSUM"))
            ps_tp = ctx.enter_context(
                tc.tile_pool(name="ps_tp", bufs=2, space="PSUM"))

            # constants split: the 3 phase-A/transpose/final mats load first
            # (tiny DMA) so the first W-stationary matmul isn't gated on the
            # full factor-matrix table; the group mats follow the first W load.
            csb1 = csb_p.tile([128, 3 * 128], F16, tag="csb1")
            csb2 = csb_p.tile([128, (n_const_mats - 3) * 128], F16,
                              tag="csb2")
            nc.sync.dma_start(csb1[:], cst[:, 0:3 * 128])

            def cmat(i):
                if i < 3:
                    return csb1[:, i * 128:(i + 1) * 128]
                return csb2[:, (i - 3) * 128:(i - 2) * 128]

            LY, LS4 = cmat(0), cmat(1)
            ident = cmat(2)
            sgn2 = cmat(n_const_mats - 1)[:, 0:2]

            def phase_a_sg(ch, t1T, sg):
                """One 4-stripe section of phase A: W-stationary S1' emits
                T1^T pieces into the consolidated tile [q-part, (qs, s, h.o)]."""
                t1Tv = t1T[:].rearrange("p (qs f) -> p qs f", qs=6)
                wt = wch_p.tile([128, 4 * CSH], F16, tag="w",
                                name=f"w_{ch}_{sg}")
                r0 = (ch * NSTR_CH + sg) * 128
                nc.sync.dma_start(
                    wt[:].rearrange("p (s c) -> p s c", s=4),
                    w[r0:r0 + 512, :].rearrange("(s p) c -> p s c", p=128))
                for qp in range(3):
                    pb = ps_2b.tile([128, 1024], F32, tag="pb", name="pb")
                    for half in range(2):
                        qs = 2 * qp + half
                        for k4 in range(4):
                            lhsT = wt[:, k4 * CSH + qs * 128:
                                      k4 * CSH + (qs + 1) * 128]
                            nc.tensor.matmul(
                                pb[:, half * 512 + k4 * 128:
                                   half * 512 + (k4 + 1) * 128], lhsT, LY)
                    ev.copy(
                        t1Tv[:, 2 * qp:2 * qp + 2, sg * 128:(sg + 4) * 128],
                        pb[:].rearrange("p (h f) -> p h f", h=2))

            def b_groups(ch, t1T, q):
                """sigma prep + fused group matmuls; returns the Z tile."""
                tvs4 = t1T[:].rearrange("p (qs b o) -> p qs b o", qs=6, o=64)
                t1s = [t1s_p.tile([128, 48 * 48], F16, tag=f"t1s{cp}",
                                  name=f"t1s_{ch}_{q}_{cp}")
                       for cp in range(2)]
                for cp in range(2):
                    src = tvs4[:, 2 * q + cp, :, 0:48].rearrange(
                        "p b (pr two) -> p b pr two", two=2)[:, :, :, ::-1]
                    sv = t1s[cp][:].rearrange(
                        "p (b pr two) -> p b pr two", pr=24, two=2)
                    sgb = sgn2.unsqueeze(1).unsqueeze(1).broadcast_to(
                        [128, 48, 24, 2])
                    nc.vector.tensor_mul(sv, src, sgb)
                    ev.t_dve += 1600.0
                zsb = zsb_p.tile([128, 2 * NSTR_CH * 128], F16,
                                 tag="z", name=f"z_{ch}_{q}")
                svs = [t1s[cp][:].rearrange("p (b o) -> p b o", o=48)
                       for cp in range(2)]
                zvs = zsb[:].rearrange("p (cp b o) -> p cp b o", cp=2, o=64)
                for g in ginfo:
                    mem = g["mem"]
                    no = 6 if g["is_R"] else 8
                    zp = ps_2b.tile([128, 1024], F32, tag="pb", name="zp")
                    zpv = zp[:].rearrange("p (cp f) -> p cp f", cp=2)
                    for cp in range(2):
                        dst = zpv[:, cp, 0:48 * no]
                        if g["is_R"]:
                            nc.tensor.matmul(
                                dst, cmat(g["gi"]),
                                tvs4[:, 2 * q + cp, :, mem[0]:mem[0] + 6],
                                start=True, stop=False)
                            nc.tensor.matmul(
                                dst, cmat(g["ji"]),
                                svs[cp][:, :, mem[0]:mem[0] + 6],
                                start=False, stop=True)
                        else:
                            nc.tensor.matmul(
                                dst, cmat(g["gi"]),
                                tvs4[:, 2 * q + cp, :, mem[0]:64:2])
                    if g["is_R"]:
                        zdst = zvs[:, :, :, mem[0]:mem[0] + 6]
                    else:
                        zdst = zvs[:, :, :, mem[0]:64:2]
                    ev.copy(zdst, zpv[:, :, 0:48 * no].rearrange(
                        "p cp (b o) -> p cp b o", o=no))
                return zsb

            def b_trans(ch, q, zsb):
                """PE-transpose Z into zt."""
                zt = ztsb_p.tile([128, NSTR_CH * 256], F16, tag="zt",
                                 name=f"zt_{ch}_{q}")
                for jp in range(0, NSTR_CH, 4):
                    pb = ps_tp.tile([128, 1024], F16, tag="tp", name="tp")
                    for k in range(8):
                        j = jp + k // 2
                        cp = k % 2
                        src = zsb[:, cp * NSTR_CH * 128 +
                                  j * 128:cp * NSTR_CH * 128 + (j + 1) * 128]
                        nc.tensor.transpose(
                            pb[:, k * 128:(k + 1) * 128], src, ident)
                    ev.copy(zt[:, jp * 256:(jp + 4) * 256], pb[:],
                            both16=True)
                return zt

            def b_final(ch, q, zt):
                """final contraction + store."""
                for jq in range(0, NSTR_CH, 4):
                    ob = osb_p.tile([128, 1024], F16, tag="ob")
                    po = ps_2b.tile([128, 1024], F32, tag="pb", name="po")
                    for h in range(2):
                        jp = jq + 2 * h
                        nc.tensor.matmul(
                            po[:, h * 512:(h + 1) * 512], LS4,
                            zt[:, jp * 256:(jp + 2) * 256])
                    ev.copy(ob[:], po[:])
                    nc.sync.dma_start(out[q, ch * 6 + jq // 4], ob[:])

            # phase A for both chunks first (W prefetch streams ahead),
            # then the six B sections in a 2-stage software pipeline:
            # finals of section i-1 fill the PE while section i's Z
            # evacuations drain.
            t1T0 = t1T_p.tile([128, 6 * NSTR_CH * 128], F16, tag="t1T",
                              name="t1T_0")
            t1T1 = t1T_p.tile([128, 6 * NSTR_CH * 128], F16, tag="t1T",
                              name="t1T_1")
            phase_a_sg(0, t1T0, 0)
            nc.sync.dma_start(csb2[:], cst[:, 3 * 128:])
            for sg in range(4, NSTR_CH, 4):
                phase_a_sg(0, t1T0, sg)
            for sg in range(0, NSTR_CH, 4):
                phase_a_sg(1, t1T1, sg)
            secs = [(0, t1T0, 0), (0, t1T0, 1), (0, t1T0, 2),
                    (1, t1T1, 0), (1, t1T1, 1), (1, t1T1, 2)]
            prev = None
            for ch, t1T, q in secs:
                zsb = b_groups(ch, t1T, q)
                if prev is not None:
                    b_final(*prev)
                zt = b_trans(ch, q, zsb)
                prev = (ch, q, zt)
            b_final(*prev)
    cap_sync_waits(nc)
    return nc


_CACHE = {}


def kernel(W, U_y, U_x, mask, block_rows, block_cols):
    from concourse import bass_utils

    W = np.asarray(W, np.float32).astype(np.float16)
    const, ginfo = host_precompute(U_y, U_x, mask, block_rows, block_cols)
    n_mats = const.shape[1] // 128

    key = ("nc", n_mats, tuple(tuple(g["mem"]) for g in ginfo))
    if key not in _CACHE:
        _CACHE[key] = build_kernel(n_mats, ginfo)
    nc = _CACHE[key]

    in_maps = []
    for core in range(NCORE):
        Wsh = np.ascontiguousarray(W[:, core * CSH:(core + 1) * CSH])
        in_maps.append({"w": Wsh, "cst": const})

    res = None
    last_exc = None
    for attempt in range(3):
        try:
            res = bass_utils.run_bass_kernel_spmd(
                nc, in_maps, core_ids=list(range(NCORE)))
            break
        except Exception as e:  # transient NRT_EXEC_UNIT states recover
            last_exc = e
            time.sleep(20 * (attempt + 1))
    if res is None:
        raise last_exc
    outs = []
    for core in range(NCORE):
        o3 = np.asarray(res.results[core]["out"], np.float32)
        o = o3.reshape(3, 2, 6, 128, 4, 256).transpose(
            1, 2, 4, 3, 0, 5).reshape(6144, CSH)
        outs.append(o)
    return np.ascontiguousarray(np.concatenate(outs, axis=1))
